# revision 2
# baseline (speedup 1.0000x reference)
"""Expert-parallel grouped-MLP (MoE experts) kernel for 8 Trainium2 cores.

Problem: y = W2_e @ silu(W1_e @ x_e + b1_e) + b2_e for E=16 independent
experts (grouped 1x1 conv), B=8 batches, C=256 channels/expert, CAP=4,
L=1024 positions.

Sharding: expert-parallel — core i owns experts {2i, 2i+1}; no cross-core
communication.

Precision strategy (fp8 DoubleRow, error-compensated): every matmul
operand is split on one side into fp8e4m3 hi+lo terms:
  A@B ~= A_hi@B_hi + A_hi@B_lo + A_lo@B_hi     (lo*lo dropped)
Each term runs as a DoubleRow fp8 matmul (256-contraction per pass at
0.5 cyc/col), so a logical 256-contraction matmul costs 1.5 cyc/col vs
fp16's 2.0 — and *every* operand is represented to ~2x fp8 mantissa, so
the result is fp16-class accurate (measured rel err ~2e-3 vs the 2e-2
gate; plain fp8 would be 4-7e-2).

Per (b, e) pair on-device:
  L1: per m-tile (8): 6 DoubleRow matmuls -> PSUM[128,512]x2,
      ACT silu(psum/16 + b1) -> h16 fp16, DVE cast -> h8 fp8,
      Pool h16-h8 -> r8 fp8 (the hi/lo split of h for layer 2).
  L2: per j-tile (2): 24 DoubleRow matmuls (4 k-pairs x 3 terms x 2 n)
      -> PSUM, ACT identity(psum/32 + b2) -> y fp16 -> DMA out.

W1 is pre-scaled by 16 and W2 by 32 on the host before fp8 splitting so
the lo residuals clear the e4m3 subnormal floor; the scales fold into
the ACT scale operand. x is pre-split hi/lo on the host (DMA ships fp8,
halving x traffic); y returns fp16 (upcast on host).
"""
import numpy as np
import ml_dtypes

import concourse.tile as tile
from concourse import bacc, mybir
from concourse.bass_utils import run_bass_kernel_spmd

# Problem constants (hardcoded per contract)
B, E, C, CAP, L = 8, 16, 256, 4, 1024
F = C * CAP            # 1024 hidden per expert
NCORES = 8
EPC = E // NCORES      # 2 experts per core
P = 128                # partitions
KI = 2                 # DoubleRow k-interleave (256-contraction)
MT = F // P            # 8 m-tiles (layer-1 output partitions)
JT = C // P            # 2 j-tiles (layer-2 output partitions)
QT = F // (KI * P)     # 4 DoubleRow k-pairs (layer-2 contraction)
NT = L // 512          # 2 n-tiles of 512 cols
N_WARMUP = 16          # dummy PE warmup matmuls
SW1, SW2 = 16.0, 32.0  # host pre-scales on W1/W2 (fold out via ACT scale)

_FP32 = mybir.dt.float32
_FP16 = mybir.dt.float16
_FP8 = mybir.dt.float8e4
_E4 = ml_dtypes.float8_e4m3
_DR = None  # set in _build (mybir.MatmulPerfMode.DoubleRow)


def _build():
    nc = bacc.Bacc("TRN2", target_bir_lowering=False, debug=False)
    DR = mybir.MatmulPerfMode.DoubleRow
    Silu = mybir.ActivationFunctionType.Silu
    Ident = mybir.ActivationFunctionType.Identity

    # host layouts (all DMA-contiguous per partition):
    #   xs[b, e, p, t, i, l]  = split_t(x[b, e, i*128+p, l])      (t: hi/lo)
    #   w1[e, t, p, i, f]     = split_t(16*W1r[e, f, i*128+p])
    #   w2[e, t, p, q, i, c]  = split_t(32*W2r[e, c, q*256+i*128+p])
    xs_d = nc.dram_tensor("xs", [B, EPC, P, 2, KI, L], _FP8, kind="ExternalInput")
    w1_d = nc.dram_tensor("w1", [EPC, 2, P, KI, F], _FP8, kind="ExternalInput")
    w2_d = nc.dram_tensor("w2", [EPC, 2, P, QT, KI, C], _FP8, kind="ExternalInput")
    b1_d = nc.dram_tensor("b1s", [EPC, F], _FP32, kind="ExternalInput")
    b2_d = nc.dram_tensor("b2s", [EPC, C], _FP32, kind="ExternalInput")
    ys_d = nc.dram_tensor("ys", [B, EPC * C, L], _FP16, kind="ExternalOutput")

    with tile.TileContext(nc) as tc:
        with (
            tc.tile_pool(name="const", bufs=1) as cpool,
            tc.tile_pool(name="x", bufs=6) as xpool,
            tc.tile_pool(name="h", bufs=2) as hpool,
            tc.tile_pool(name="y", bufs=4) as ypool,
            tc.tile_pool(name="ps1", bufs=4, space="PSUM") as ps1,
            tc.tile_pool(name="ps2", bufs=4, space="PSUM") as ps2,
        ):
            # ---- PE warmup: zero bf16 matmuls with no DMA deps ----
            wdum = cpool.tile([P, P], mybir.dt.bfloat16, tag="wdum")
            rdum = cpool.tile([P, 512], mybir.dt.bfloat16, tag="rdum")
            nc.vector.memset(wdum[:], 0.0)
            nc.vector.memset(rdum[:], 0.0)
            actdum = cpool.tile([P, 1], _FP32, tag="actdum")
            nc.scalar.activation(actdum[:], rdum[:, :1], Silu, bias=0.0)
            for i in range(N_WARMUP):
                pdum = ps1.tile([P, 512], _FP32, tag="ps1")
                nc.tensor.matmul(pdum[:], wdum[:], rdum[:],
                                 start=True, stop=True)

            # ---- weight/bias tiles ----
            # w1sb[e][t]: [P, KI, F];  w2sb[e][t]: [P, QT, KI, C]
            w1sb = [[cpool.tile([P, KI, F], _FP8, tag=f"w1_{e}_{t}",
                                name=f"w1sb_{e}_{t}") for t in range(2)]
                    for e in range(EPC)]
            w2sb = [[cpool.tile([P, QT, KI, C], _FP8, tag=f"w2_{e}_{t}",
                                name=f"w2sb_{e}_{t}") for t in range(2)]
                    for e in range(EPC)]
            b1sb = cpool.tile([P, EPC * MT], _FP32, tag="b1")  # col e*MT+m
            b2sb = cpool.tile([P, EPC * JT], _FP32, tag="b2")  # col e*JT+j

            def load_w(e):
                for t in range(2):
                    nc.sync.dma_start(w1sb[e][t][:], w1_d.ap()[e, t])
                    nc.sync.dma_start(w2sb[e][t][:], w2_d.ap()[e, t])

            def load_b(e):
                nc.sync.dma_start(
                    b1sb[:, e * MT:(e + 1) * MT],
                    b1_d.ap()[e].rearrange("(m p) -> p m", p=P),
                )
                nc.sync.dma_start(
                    b2sb[:, e * JT:(e + 1) * JT],
                    b2_d.ap()[e].rearrange("(j p) -> p j", p=P),
                )

            def load_x(b, e):
                # [P, 2(hilo), KI, L] fp8, contiguous 4KB/partition slab
                xt = xpool.tile([P, 2, KI, L], _FP8, tag="x",
                                name=f"x_{b}_{e}")
                nc.sync.dma_start(xt[:, 0], xs_d.ap()[b, e, :, 0])
                nc.sync.dma_start(xt[:, 1], xs_d.ap()[b, e, :, 1])
                return xt

            # startup-critical order: expert-0 weights + pair-0 x first;
            # expert-1 weights deferred until early x prefetches queued
            load_b(0)
            nc.sync.dma_start(w1sb[0][0][:], w1_d.ap()[0, 0])
            x0 = load_x(0, 0)
            nc.sync.dma_start(w1sb[0][1][:], w1_d.ap()[0, 1])
            nc.sync.dma_start(w2sb[0][0][:], w2_d.ap()[0, 0])
            nc.sync.dma_start(w2sb[0][1][:], w2_d.ap()[0, 1])

            # ---- per-(expert, batch) pipeline ----
            for e in range(EPC):
                for b in range(B):
                    xt = x0 if (e == 0 and b == 0) else load_x(b, e)
                    if e == 0 and b == 1:
                        load_w(1)
                        load_b(1)

                    # layer 1: h = silu((W1s@x)/16 + b1), 3-term fp8
                    h16 = hpool.tile([P, MT, L], _FP16, tag="h16",
                                     name=f"h16_{e}_{b}")
                    h8 = hpool.tile([P, MT, L], _FP8, tag="h8",
                                    name=f"h8_{e}_{b}")
                    r8 = hpool.tile([P, MT, L], _FP8, tag="r8",
                                    name=f"r8_{e}_{b}")
                    for m in range(MT):
                        lhi = w1sb[e][0][:, :, m * P:(m + 1) * P]
                        llo = w1sb[e][1][:, :, m * P:(m + 1) * P]
                        psm = [ps1.tile([P, 512], _FP32, tag="ps1",
                                        name=f"ps1_{e}_{b}_{m}_{n}")
                               for n in range(NT)]
                        for n in range(NT):  # hi*hi (stationary lhi)
                            nc.tensor.matmul(
                                psm[n][:], lhi, xt[:, 0, :, n * 512:(n + 1) * 512],
                                start=True, stop=False, perf_mode=DR)
                        for n in range(NT):  # hi*lo (stationary lhi reused)
                            nc.tensor.matmul(
                                psm[n][:], lhi, xt[:, 1, :, n * 512:(n + 1) * 512],
                                start=False, stop=False, perf_mode=DR)
                        for n in range(NT):  # lo*hi
                            nc.tensor.matmul(
                                psm[n][:], llo, xt[:, 0, :, n * 512:(n + 1) * 512],
                                start=False, stop=True, perf_mode=DR)
                        for n in range(NT):
                            nc.scalar.activation(
                                h16[:, m, n * 512:(n + 1) * 512], psm[n][:],
                                Silu,
                                bias=b1sb[:, e * MT + m: e * MT + m + 1],
                                scale=1.0 / SW1)
                        # hi/lo split of h for layer 2
                        nc.vector.tensor_copy(h8[:, m], h16[:, m])
                        nc.gpsimd.tensor_sub(r8[:, m], h16[:, m], h8[:, m])

                    # layer 2: y = (W2s@(h8+r8))/32 + b2
                    for j in range(JT):
                        psy = [ps2.tile([P, 512], _FP32, tag="ps2",
                                        name=f"ps2_{e}_{b}_{j}_{n}")
                               for n in range(NT)]
                        for q in range(QT):
                            lhi = w2sb[e][0][:, q, :, j * P:(j + 1) * P]
                            llo = w2sb[e][1][:, q, :, j * P:(j + 1) * P]
                            for n in range(NT):  # hi*h8
                                nc.tensor.matmul(
                                    psy[n][:], lhi,
                                    h8[:, 2 * q:2 * q + 2, n * 512:(n + 1) * 512],
                                    start=(q == 0), stop=False, perf_mode=DR)
                            for n in range(NT):  # hi*r8 (stationary reused)
                                nc.tensor.matmul(
                                    psy[n][:], lhi,
                                    r8[:, 2 * q:2 * q + 2, n * 512:(n + 1) * 512],
                                    start=False, stop=False, perf_mode=DR)
                            for n in range(NT):  # lo*h8
                                nc.tensor.matmul(
                                    psy[n][:], llo,
                                    h8[:, 2 * q:2 * q + 2, n * 512:(n + 1) * 512],
                                    start=False, stop=(q == QT - 1),
                                    perf_mode=DR)
                        for n in range(NT):
                            yt = ypool.tile([P, 512], _FP16, tag="y",
                                            name=f"y_{e}_{b}_{j}_{n}")
                            nc.scalar.activation(
                                yt[:], psy[n][:], Ident,
                                bias=b2sb[:, e * JT + j: e * JT + j + 1],
                                scale=1.0 / SW2)
                            nc.sync.dma_start(
                                ys_d.ap()[b, e * C + j * P: e * C + (j + 1) * P,
                                          n * 512:(n + 1) * 512],
                                yt[:])

    nc.compile()
    return nc


_NC_CACHE = None


def _get_nc():
    global _NC_CACHE
    if _NC_CACHE is None:
        _NC_CACHE = _build()
    return _NC_CACHE


def _split8(a):
    """fp8e4m3 hi/lo split: a ~= hi + lo (both e4m3)."""
    hi = a.astype(_E4)
    lo = (a - hi.astype(np.float32)).astype(_E4)
    return hi, lo


def _shard_inputs(x, W1, b1, W2, b2):
    """Full inputs -> list of 8 per-core input dicts (expert-parallel)."""
    x = np.ascontiguousarray(x, dtype=np.float32)
    # x: [B, E*C, L] -> [B, E, KI, P, L] -> hi/lo -> [B, E, P, 2, KI, L]
    xr = x.reshape(B, E, KI, P, L)
    xhi, xlo = _split8(xr)
    xs = np.stack([xhi, xlo], axis=2)              # [B, E, 2, KI, P, L]
    xs = np.ascontiguousarray(xs.transpose(0, 1, 4, 2, 3, 5))

    # W1: [E*F, C] -> W1r [E, F, C]; lhsT = W1r.T [C, F] scaled by 16
    w1t = W1.astype(np.float32).reshape(E, F, C).transpose(0, 2, 1) * SW1
    w1t = w1t.reshape(E, KI, P, F)                  # [E, i, p, f]
    w1hi, w1lo = _split8(w1t)
    w1s = np.stack([w1hi, w1lo], axis=1)            # [E, 2, i, p, f]
    w1s = np.ascontiguousarray(w1s.transpose(0, 1, 3, 2, 4))  # [E,2,p,i,f]

    # W2: [E*C, F] -> W2r [E, C, F]; lhsT = W2r.T [F, C] scaled by 32
    w2t = W2.astype(np.float32).reshape(E, C, F).transpose(0, 2, 1) * SW2
    w2t = w2t.reshape(E, QT, KI, P, C)              # [E, q, i, p, c]
    w2hi, w2lo = _split8(w2t)
    w2s = np.stack([w2hi, w2lo], axis=1)            # [E, 2, q, i, p, c]
    w2s = np.ascontiguousarray(w2s.transpose(0, 1, 4, 2, 3, 5))  # [E,2,p,q,i,c]

    b1r = np.ascontiguousarray(b1.astype(np.float32).reshape(E, F))
    b2r = np.ascontiguousarray(b2.astype(np.float32).reshape(E, C))

    in_maps = []
    for i in range(NCORES):
        es = slice(i * EPC, (i + 1) * EPC)
        in_maps.append({
            "xs": np.ascontiguousarray(xs[:, es]),
            "w1": np.ascontiguousarray(w1s[es]),
            "w2": np.ascontiguousarray(w2s[es]),
            "b1s": b1r[es],
            "b2s": b2r[es],
        })
    return in_maps


def run(x, W1, b1, W2, b2, trace=False, **trace_kwargs):
    nc = _get_nc()
    in_maps = _shard_inputs(x, W1, b1, W2, b2)
    res = run_bass_kernel_spmd(
        nc, in_maps, core_ids=list(range(NCORES)), trace=trace, **trace_kwargs
    )
    y = np.concatenate([res.results[i]["ys"] for i in range(NCORES)], axis=1)
    return y.astype(np.float32), res


def kernel(x, W1, b1, W2, b2):
    y, _ = run(x, W1, b1, W2, b2)
    return y


# revision 8
# speedup vs baseline: 1.3611x; 1.3611x over previous
"""Expert-parallel grouped-MLP (MoE experts) kernel for 8 Trainium2 cores.

Problem: y = W2_e @ silu(W1_e @ x_e + b1_e) + b2_e for E=16 independent
experts (grouped 1x1 conv), B=8 batches, C=256 channels/expert, CAP=4,
L=1024 positions. Expert-parallel: core i owns experts {2i, 2i+1}.

Speed trick ("linear hoist + single-pass fp8 residual path"):
  silu(z) = 0.5*z + g(z),  g = silu(z) - 0.5*z  (sigma_g ~ 0.45*sigma_h)
  y = W2@g + Wf@x + b2,    Wf := 0.5*(W2@W1)  (fused [C,C], host-exact)
The g-path runs as SINGLE fp8e4m3 DoubleRow matmuls (256-contraction per
pass -> 2x fp16 FLOP rate); g's small amplitude keeps the fp8
quantization error of both g and W2 inside the 2e-2 gate (measured
1.53e-2 on the fixed seed-0 inputs; plain fp8 h-path would be 5.3e-2).
The f-path and layer 1 stay fp16/exact. 28 512-col PE passes per
(pair, n-half) vs 32 for pure fp16.

Per (b, e) pair on-device:
  L1: per m-tile (8): 4 fp16 matmuls -> psum1 [128,1024] (= 0.5*z)
      ACT: h16 = silu(2*psum1 + b1)
      DVE/GpSimd (alternating): s8 = (h16 - 0.25) - psum1  -> fp8 (= g-0.25)
  L2: per (j,n): 2 fp16 Wf-matmuls + 4 fp8-DR W2g-matmuls -> psum2
      DVE: y16 = psum2/32 + b2'   (b2' = b2 + 0.25*rowsum(W2), host)
Host pre-scales: W1 x0.5 (psum holds 0.5z), W2g x32 fp8, Wf x16 fp16;
x ships fp16, y returns fp16 (upcast on host).
"""
import numpy as np
import ml_dtypes

import concourse.tile as tile
from concourse import bacc, mybir
from concourse.bass_utils import run_bass_kernel_spmd

# Problem constants (hardcoded per contract)
B, E, C, CAP, L = 8, 16, 256, 4, 1024
F = C * CAP            # 1024 hidden per expert
NCORES = 8
EPC = E // NCORES      # 2 experts per core
P = 128                # partitions
KT = C // P            # 2 fp16 k-tiles (layer-1 / f-path contraction)
KI = 2                 # DoubleRow k-interleave (256-contraction)
MT = F // P            # 8 m-tiles
JT = C // P            # 2 j-tiles
QT = F // (KI * P)     # 4 DoubleRow k-pairs (g-path contraction)
NT = L // 512          # 2 n-tiles of 512 cols
N_WARMUP = 16
SW = 32.0              # W2 scale
SHIFT = 0.25           # g mean shift (folded into b2')
GPS_M = 3              # m-tiles whose s8 runs via ACT-evac + GpSimd sub

_FP32 = mybir.dt.float32
_FP16 = mybir.dt.float16
_FP8 = mybir.dt.float8e4
_E4 = ml_dtypes.float8_e4m3


def _build():
    nc = bacc.Bacc("TRN2", target_bir_lowering=False, debug=False)
    DR = mybir.MatmulPerfMode.DoubleRow
    Silu = mybir.ActivationFunctionType.Silu
    Ident = mybir.ActivationFunctionType.Identity
    Sub = mybir.AluOpType.subtract
    Mult = mybir.AluOpType.mult
    Add = mybir.AluOpType.add

    # host layouts (contiguous per partition):
    #   xf[b, e, p, k, l] = fp16(x[b, e, k*128+p, l])
    #   w1[e, p, k, f]    = fp16(0.5 * W1r[e, f, k*128+p])
    #   wf[e, p, k, c]    = fp16(16 * (W2r@W1r)[e, c, k*128+p])
    #   w2[e, p, q, i, c] = fp8(32 * W2r[e, c, q*256+i*128+p])
    xs_d = nc.dram_tensor("xs", [B, EPC, P, KT, L], _FP16, kind="ExternalInput")
    w1_d = nc.dram_tensor("w1", [EPC, P, KT, F], _FP16, kind="ExternalInput")
    wf_d = nc.dram_tensor("wf", [EPC, P, KT, C], _FP16, kind="ExternalInput")
    w2_d = nc.dram_tensor("w2", [EPC, P, QT, KI, C], _FP8, kind="ExternalInput")
    b1_d = nc.dram_tensor("b1s", [EPC, F], _FP32, kind="ExternalInput")
    b2_d = nc.dram_tensor("b2s", [EPC, C], _FP32, kind="ExternalInput")
    ys_d = nc.dram_tensor("ys", [B, EPC * C, L], _FP16, kind="ExternalOutput")

    with tile.TileContext(nc) as tc:
        with (
            tc.tile_pool(name="const", bufs=1) as cpool,
            tc.tile_pool(name="x", bufs=6) as xpool,
            tc.tile_pool(name="h", bufs=2) as hpool,
            tc.tile_pool(name="y", bufs=4) as ypool,
            tc.tile_pool(name="ps1", bufs=2, space="PSUM") as ps1,
            tc.tile_pool(name="ps2", bufs=4, space="PSUM") as ps2,
        ):
            # ---- PE warmup: zero bf16 matmuls with no DMA deps ----
            wdum = cpool.tile([P, P], mybir.dt.bfloat16, tag="wdum")
            rdum = cpool.tile([P, 512], mybir.dt.bfloat16, tag="rdum")
            nc.vector.memset(wdum[:], 0.0)
            nc.vector.memset(rdum[:], 0.0)
            actdum = cpool.tile([P, 1], _FP32, tag="actdum")
            nc.scalar.activation(actdum[:], rdum[:, :1], Silu, bias=0.0)
            shiftc = cpool.tile([P, 1], _FP32, tag="shiftc")
            nc.vector.memset(shiftc[:], SHIFT)
            for i in range(N_WARMUP):
                pdum = ps2.tile([P, 512], _FP32, tag="ps2")
                nc.tensor.matmul(pdum[:], wdum[:], rdum[:],
                                 start=True, stop=True)

            # ---- weight/bias tiles ----
            w1sb = [cpool.tile([P, KT, F], _FP16, tag=f"w1_{e}",
                               name=f"w1sb_{e}") for e in range(EPC)]
            wfsb = [cpool.tile([P, KT, C], _FP16, tag=f"wf_{e}",
                               name=f"wfsb_{e}") for e in range(EPC)]
            w2sb = [cpool.tile([P, QT, KI, C], _FP8, tag=f"w2_{e}",
                               name=f"w2sb_{e}") for e in range(EPC)]
            b1sb = cpool.tile([P, EPC * MT], _FP32, tag="b1")  # col e*MT+m
            b2sb = cpool.tile([P, EPC * JT], _FP32, tag="b2")  # col e*JT+j

            def load_w(e):
                nc.sync.dma_start(w1sb[e][:], w1_d.ap()[e])
                nc.sync.dma_start(wfsb[e][:], wf_d.ap()[e])
                nc.sync.dma_start(w2sb[e][:], w2_d.ap()[e])

            def load_b(e):
                nc.sync.dma_start(
                    b1sb[:, e * MT:(e + 1) * MT],
                    b1_d.ap()[e].rearrange("(m p) -> p m", p=P),
                )
                nc.sync.dma_start(
                    b2sb[:, e * JT:(e + 1) * JT],
                    b2_d.ap()[e].rearrange("(j p) -> p j", p=P),
                )

            def load_x(b, e):
                # [P, KT, L] fp16; split per k-tile so mm0 waits on 2KB/part
                xt = xpool.tile([P, KT, L], _FP16, tag="x", name=f"x_{b}_{e}")
                for k in range(KT):
                    nc.sync.dma_start(xt[:, k], xs_d.ap()[b, e, :, k])
                return xt

            # startup-critical order
            load_b(0)
            nc.sync.dma_start(w1sb[0][:], w1_d.ap()[0])
            x0 = load_x(0, 0)
            nc.sync.dma_start(wfsb[0][:], wf_d.ap()[0])
            nc.sync.dma_start(w2sb[0][:], w2_d.ap()[0])

            # ---- per-(expert, batch) pipeline ----
            for e in range(EPC):
                for b in range(B):
                    xt = x0 if (e == 0 and b == 0) else load_x(b, e)
                    if e == 0 and b == 1:
                        load_w(1)
                        load_b(1)

                    # layer 1: psum1 = 0.5*z; h16 = silu(2*psum1 + b1)
                    h16 = hpool.tile([P, MT, L], _FP16, tag="h16",
                                     name=f"h16_{e}_{b}")
                    s8 = hpool.tile([P, MT, L], _FP8, tag="s8",
                                    name=f"s8_{e}_{b}")
                    for m in range(MT):
                        psm = ps1.tile([P, L], _FP32, tag="ps1",
                                       name=f"ps1_{e}_{b}_{m}")
                        for k in range(KT):
                            for n in range(NT):
                                nc.tensor.matmul(
                                    psm[:, n * 512:(n + 1) * 512],
                                    w1sb[e][:, k, m * P:(m + 1) * P],
                                    xt[:, k, n * 512:(n + 1) * 512],
                                    start=(k == 0), stop=(k == KT - 1))
                        nc.scalar.activation(
                            h16[:, m], psm[:], Silu,
                            bias=b1sb[:, e * MT + m: e * MT + m + 1],
                            scale=2.0)
                        # s8 = (h16 - SHIFT) - psum1   (= g - SHIFT, fp8)
                        if m < GPS_M:
                            # GpSimd can't read PSUM: ACT evacuates
                            # t16 = psum + SHIFT, GpSimd subtracts in SBUF
                            t16 = hpool.tile([P, L], _FP16, tag=f"t{m}",
                                             name=f"t16_{e}_{b}_{m}")
                            nc.scalar.activation(t16[:], psm[:], Ident,
                                                 bias=shiftc[:, 0:1], scale=1.0)
                            nc.gpsimd.tensor_sub(s8[:, m], h16[:, m], t16[:])
                        else:
                            nc.vector.scalar_tensor_tensor(
                                s8[:, m], h16[:, m], SHIFT, psm[:],
                                op0=Sub, op1=Sub)

                    # layer 2: psum2 = Wf@x (fp16) + W2g@s8 (fp8 DR)
                    for j in range(JT):
                        psy = [ps2.tile([P, 512], _FP32, tag="ps2",
                                        name=f"ps2_{e}_{b}_{j}_{n}")
                               for n in range(NT)]
                        for k in range(KT):
                            for n in range(NT):
                                nc.tensor.matmul(
                                    psy[n][:],
                                    wfsb[e][:, k, j * P:(j + 1) * P],
                                    xt[:, k, n * 512:(n + 1) * 512],
                                    start=(k == 0), stop=False)
                        for q in range(QT):
                            for n in range(NT):
                                nc.tensor.matmul(
                                    psy[n][:],
                                    w2sb[e][:, q, :, j * P:(j + 1) * P],
                                    s8[:, 2 * q:2 * q + 2,
                                       n * 512:(n + 1) * 512],
                                    start=False, stop=(q == QT - 1),
                                    perf_mode=DR)
                        for n in range(NT):
                            yt = ypool.tile([P, 512], _FP16, tag="y",
                                            name=f"y_{e}_{b}_{j}_{n}")
                            nc.vector.tensor_scalar(
                                yt[:], psy[n][:], 1.0 / SW,
                                b2sb[:, e * JT + j: e * JT + j + 1],
                                op0=Mult, op1=Add)
                            nc.sync.dma_start(
                                ys_d.ap()[b, e * C + j * P: e * C + (j + 1) * P,
                                          n * 512:(n + 1) * 512],
                                yt[:])

    nc.compile()
    return nc


_NC_CACHE = None


def _get_nc():
    global _NC_CACHE
    if _NC_CACHE is None:
        _NC_CACHE = _build()
    return _NC_CACHE


def _shard_inputs(x, W1, b1, W2, b2):
    """Full inputs -> list of 8 per-core input dicts (expert-parallel)."""
    x = np.ascontiguousarray(x, dtype=np.float32)
    # xf[b, e, p, k, l]
    xf = np.ascontiguousarray(
        x.reshape(B, E, KT, P, L).transpose(0, 1, 3, 2, 4).astype(np.float16))

    W1r = W1.astype(np.float32).reshape(E, F, C)
    W2r = W2.astype(np.float32).reshape(E, C, F)
    b1r = b1.astype(np.float32).reshape(E, F)
    b2r = b2.astype(np.float32).reshape(E, C)

    # w1[e, p, k, f] = 0.5 * W1r[e].T, fp16
    w1t = (0.5 * W1r).transpose(0, 2, 1).reshape(E, KT, P, F)
    w1s = np.ascontiguousarray(w1t.transpose(0, 2, 1, 3).astype(np.float16))
    # wf[e, p, k, c] = 16 * (W2r@W1r)[e].T, fp16
    wfr = 16.0 * np.einsum('ecf,efd->ecd', W2r, W1r, optimize=True)  # [E,C,C]
    wft = wfr.transpose(0, 2, 1).reshape(E, KT, P, C)
    wfs = np.ascontiguousarray(wft.transpose(0, 2, 1, 3).astype(np.float16))
    # w2[e, p, q, i, c] = fp8(32 * W2r[e].T)
    w2t = (SW * W2r).transpose(0, 2, 1).reshape(E, QT, KI, P, C)
    w2s = np.ascontiguousarray(w2t.transpose(0, 3, 1, 2, 4).astype(_E4))
    # b2' = b2 + SHIFT * rowsum(W2)
    b2p = np.ascontiguousarray(b2r + SHIFT * W2r.sum(axis=2))
    b1c = np.ascontiguousarray(b1r)

    in_maps = []
    for i in range(NCORES):
        es = slice(i * EPC, (i + 1) * EPC)
        in_maps.append({
            "xs": np.ascontiguousarray(xf[:, es]),
            "w1": np.ascontiguousarray(w1s[es]),
            "wf": np.ascontiguousarray(wfs[es]),
            "w2": np.ascontiguousarray(w2s[es]),
            "b1s": b1c[es],
            "b2s": b2p[es],
        })
    return in_maps


def run(x, W1, b1, W2, b2, trace=False, **trace_kwargs):
    nc = _get_nc()
    in_maps = _shard_inputs(x, W1, b1, W2, b2)
    res = run_bass_kernel_spmd(
        nc, in_maps, core_ids=list(range(NCORES)), trace=trace, **trace_kwargs
    )
    y = np.concatenate([res.results[i]["ys"] for i in range(NCORES)], axis=1)
    return y.astype(np.float32), res


def kernel(x, W1, b1, W2, b2):
    y, _ = run(x, W1, b1, W2, b2)
    return y


# revision 10
# speedup vs baseline: 1.6186x; 1.1891x over previous
"""Expert-parallel grouped-MLP (MoE experts) kernel for 8 Trainium2 cores.

Problem: y = W2_e @ silu(W1_e @ x_e + b1_e) + b2_e for E=16 independent
experts (grouped 1x1 conv), B=8 batches, C=256 channels/expert, CAP=4,
L=1024 positions. Expert-parallel: core i owns experts {2i, 2i+1}.

Speed trick ("linear hoist + single-pass fp8 residual path"):
  silu(z) = 0.5*z + g(z),  g = silu(z) - 0.5*z  (sigma_g ~ 0.45*sigma_h)
  y = W2@g + Wf@x + b2,    Wf := 0.5*(W2@W1)  (fused [C,C], host-exact)
The g-path runs as SINGLE fp8e4m3 DoubleRow matmuls (256-contraction per
pass -> 2x fp16 FLOP rate); g's small amplitude keeps the fp8
quantization error of both g and W2 inside the 2e-2 gate (measured
1.53e-2 on the fixed seed-0 inputs; plain fp8 h-path would be 5.3e-2).
The f-path and layer 1 stay fp16/exact. 28 512-col PE passes per
(pair, n-half) vs 32 for pure fp16.

Per (b, e) pair on-device:
  L1: per m-tile (8): 4 fp16 matmuls -> psum1 [128,1024] (= 0.5*z)
      ACT: h16 = silu(2*psum1 + b1)
      DVE/GpSimd (alternating): s8 = (h16 - 0.25) - psum1  -> fp8 (= g-0.25)
  L2: per (j,n): 2 fp16 Wf-matmuls + 4 fp8-DR W2g-matmuls -> psum2
      DVE: y16 = psum2/32 + b2'   (b2' = b2 + 0.25*rowsum(W2), host)
Host pre-scales: W1 x0.5 (psum holds 0.5z), W2g x32 fp8, Wf x16 fp16;
x ships fp16, y returns fp16 (upcast on host).
"""
import numpy as np
import ml_dtypes

import concourse.tile as tile
from concourse import bacc, mybir
from concourse.bass_utils import run_bass_kernel_spmd

# Problem constants (hardcoded per contract)
B, E, C, CAP, L = 8, 16, 256, 4, 1024
F = C * CAP            # 1024 hidden per expert
NCORES = 8
EPC = E // NCORES      # 2 experts per core
P = 128                # partitions
KT = C // P            # 2 fp16 k-tiles (layer-1 / f-path contraction)
KI = 2                 # DoubleRow k-interleave (256-contraction)
MT = F // P            # 8 m-tiles
JT = C // P            # 2 j-tiles
QT = F // (KI * P)     # 4 DoubleRow k-pairs (g-path contraction)
NT = L // 512          # 2 n-tiles of 512 cols
N_WARMUP = 16
SW = 32.0              # W2 scale
SHIFT = 0.25           # g mean shift (folded into b2')
GPS_M = 3              # m-tiles whose s8 runs via ACT-evac + GpSimd sub

_FP32 = mybir.dt.float32
_FP16 = mybir.dt.float16
_FP8 = mybir.dt.float8e4
_E4 = ml_dtypes.float8_e4m3


def _build():
    nc = bacc.Bacc("TRN2", target_bir_lowering=False, debug=False)
    DR = mybir.MatmulPerfMode.DoubleRow
    Silu = mybir.ActivationFunctionType.Silu
    Ident = mybir.ActivationFunctionType.Identity
    Sub = mybir.AluOpType.subtract
    Mult = mybir.AluOpType.mult
    Add = mybir.AluOpType.add

    # host layouts (contiguous per partition):
    #   xf[b, e, p, k, l] = fp16(x[b, e, k*128+p, l])
    #   w1[e, p, k, f]    = fp16(0.5 * W1r[e, f, k*128+p])
    #   wf[e, p, k, c]    = fp16(16 * (W2r@W1r)[e, c, k*128+p])
    #   w2[e, p, q, i, c] = fp8(32 * W2r[e, c, q*256+i*128+p])
    xs_d = nc.dram_tensor("xs", [B, EPC, P, KT, L], _FP16, kind="ExternalInput")
    w1_d = nc.dram_tensor("w1", [EPC, P, KT, F], _FP16, kind="ExternalInput")
    wf_d = nc.dram_tensor("wf", [EPC, P, KT, C], _FP16, kind="ExternalInput")
    w2_d = nc.dram_tensor("w2", [EPC, P, QT, KI, C], _FP8, kind="ExternalInput")
    b1_d = nc.dram_tensor("b1s", [EPC, F], _FP32, kind="ExternalInput")
    b2_d = nc.dram_tensor("b2s", [EPC, C], _FP32, kind="ExternalInput")
    ys_d = nc.dram_tensor("ys", [B, EPC * C, L], _FP16, kind="ExternalOutput")

    with tile.TileContext(nc) as tc:
        with (
            tc.tile_pool(name="const", bufs=1) as cpool,
            tc.tile_pool(name="x", bufs=6) as xpool,
            tc.tile_pool(name="h", bufs=2) as hpool,
            tc.tile_pool(name="y", bufs=4) as ypool,
            tc.tile_pool(name="ps1", bufs=3, space="PSUM") as ps1,
            tc.tile_pool(name="ps2", bufs=2, space="PSUM") as ps2,
        ):
            # ---- PE warmup: zero bf16 matmuls with no DMA deps ----
            wdum = cpool.tile([P, P], mybir.dt.bfloat16, tag="wdum")
            rdum = cpool.tile([P, 512], mybir.dt.bfloat16, tag="rdum")
            nc.vector.memset(wdum[:], 0.0)
            nc.vector.memset(rdum[:], 0.0)
            actdum = cpool.tile([P, 1], _FP32, tag="actdum")
            nc.scalar.activation(actdum[:], rdum[:, :1], Silu, bias=0.0)
            shiftc = cpool.tile([P, 1], _FP32, tag="shiftc")
            nc.vector.memset(shiftc[:], SHIFT)
            for i in range(N_WARMUP):
                pdum = ps2.tile([P, 512], _FP32, tag="ps2")
                nc.tensor.matmul(pdum[:], wdum[:], rdum[:],
                                 start=True, stop=True)

            # ---- weight/bias tiles ----
            w1sb = [cpool.tile([P, KT, F], _FP16, tag=f"w1_{e}",
                               name=f"w1sb_{e}") for e in range(EPC)]
            wfsb = [cpool.tile([P, KT, C], _FP16, tag=f"wf_{e}",
                               name=f"wfsb_{e}") for e in range(EPC)]
            w2sb = [cpool.tile([P, QT, KI, C], _FP8, tag=f"w2_{e}",
                               name=f"w2sb_{e}") for e in range(EPC)]
            b1sb = cpool.tile([P, EPC * MT], _FP32, tag="b1")  # col e*MT+m
            b2sb = cpool.tile([P, EPC * JT], _FP32, tag="b2")  # col e*JT+j

            def load_w(e):
                nc.sync.dma_start(w1sb[e][:], w1_d.ap()[e])
                nc.sync.dma_start(wfsb[e][:], wf_d.ap()[e])
                nc.sync.dma_start(w2sb[e][:], w2_d.ap()[e])

            def load_b(e):
                nc.sync.dma_start(
                    b1sb[:, e * MT:(e + 1) * MT],
                    b1_d.ap()[e].rearrange("(m p) -> p m", p=P),
                )
                nc.sync.dma_start(
                    b2sb[:, e * JT:(e + 1) * JT],
                    b2_d.ap()[e].rearrange("(j p) -> p j", p=P),
                )

            def load_x(b, e):
                # [P, KT, L] fp16; split per k-tile so mm0 waits on 2KB/part
                xt = xpool.tile([P, KT, L], _FP16, tag="x", name=f"x_{b}_{e}")
                for k in range(KT):
                    nc.sync.dma_start(xt[:, k], xs_d.ap()[b, e, :, k])
                return xt

            # startup-critical order
            load_b(0)
            nc.sync.dma_start(w1sb[0][:], w1_d.ap()[0])
            x0 = load_x(0, 0)
            nc.sync.dma_start(wfsb[0][:], wf_d.ap()[0])
            nc.sync.dma_start(w2sb[0][:], w2_d.ap()[0])

            # ---- per-(expert, batch) pipeline ----
            for e in range(EPC):
                for b in range(B):
                    xt = x0 if (e == 0 and b == 0) else load_x(b, e)
                    if e == 0 and b == 1:
                        load_w(1)
                        load_b(1)

                    # layer 1: psum1 = 0.5*z; h16 = silu(2*psum1 + b1)
                    h16 = hpool.tile([P, MT, L], _FP16, tag="h16",
                                     name=f"h16_{e}_{b}")
                    s8 = hpool.tile([P, MT, L], _FP8, tag="s8",
                                    name=f"s8_{e}_{b}")
                    for m in range(MT):
                        psm = ps1.tile([P, L], _FP32, tag="ps1",
                                       name=f"ps1_{e}_{b}_{m}")
                        for k in range(KT):
                            for n in range(NT):
                                nc.tensor.matmul(
                                    psm[:, n * 512:(n + 1) * 512],
                                    w1sb[e][:, k, m * P:(m + 1) * P],
                                    xt[:, k, n * 512:(n + 1) * 512],
                                    start=(k == 0), stop=(k == KT - 1))
                        nc.scalar.activation(
                            h16[:, m], psm[:], Silu,
                            bias=b1sb[:, e * MT + m: e * MT + m + 1],
                            scale=2.0)
                        # s8 = (h16 - SHIFT) - psum1   (= g - SHIFT, fp8)
                        if m < GPS_M:
                            # GpSimd can't read PSUM: DVE evacuates
                            # t16 = psum + SHIFT, GpSimd subtracts in SBUF
                            t16 = hpool.tile([P, L], _FP16, tag=f"t{m}",
                                             name=f"t16_{e}_{b}_{m}")
                            nc.vector.tensor_scalar(t16[:], psm[:],
                                                    float(SHIFT), None, op0=Add)
                            nc.gpsimd.tensor_sub(s8[:, m], h16[:, m], t16[:])
                        else:
                            nc.vector.scalar_tensor_tensor(
                                s8[:, m], h16[:, m], SHIFT, psm[:],
                                op0=Sub, op1=Sub)

                    # layer 2: psum2 = Wf@x (fp16) + W2g@s8 (fp8 DR)
                    for j in range(JT):
                        psy = [ps2.tile([P, 512], _FP32, tag="ps2",
                                        name=f"ps2_{e}_{b}_{j}_{n}")
                               for n in range(NT)]
                        for k in range(KT):
                            for n in range(NT):
                                nc.tensor.matmul(
                                    psy[n][:],
                                    wfsb[e][:, k, j * P:(j + 1) * P],
                                    xt[:, k, n * 512:(n + 1) * 512],
                                    start=(k == 0), stop=False)
                        for q in range(QT):
                            for n in range(NT):
                                nc.tensor.matmul(
                                    psy[n][:],
                                    w2sb[e][:, q, :, j * P:(j + 1) * P],
                                    s8[:, 2 * q:2 * q + 2,
                                       n * 512:(n + 1) * 512],
                                    start=False, stop=(q == QT - 1),
                                    perf_mode=DR)
                        for n in range(NT):
                            yt = ypool.tile([P, 512], _FP16, tag="y",
                                            name=f"y_{e}_{b}_{j}_{n}")
                            nc.vector.tensor_scalar(
                                yt[:], psy[n][:], 1.0 / SW,
                                b2sb[:, e * JT + j: e * JT + j + 1],
                                op0=Mult, op1=Add)
                            nc.sync.dma_start(
                                ys_d.ap()[b, e * C + j * P: e * C + (j + 1) * P,
                                          n * 512:(n + 1) * 512],
                                yt[:])

    nc.compile()
    return nc


_NC_CACHE = None


def _get_nc():
    global _NC_CACHE
    if _NC_CACHE is None:
        _NC_CACHE = _build()
    return _NC_CACHE


def _shard_inputs(x, W1, b1, W2, b2):
    """Full inputs -> list of 8 per-core input dicts (expert-parallel)."""
    x = np.ascontiguousarray(x, dtype=np.float32)
    # xf[b, e, p, k, l]
    xf = np.ascontiguousarray(
        x.reshape(B, E, KT, P, L).transpose(0, 1, 3, 2, 4).astype(np.float16))

    W1r = W1.astype(np.float32).reshape(E, F, C)
    W2r = W2.astype(np.float32).reshape(E, C, F)
    b1r = b1.astype(np.float32).reshape(E, F)
    b2r = b2.astype(np.float32).reshape(E, C)

    # w1[e, p, k, f] = 0.5 * W1r[e].T, fp16
    w1t = (0.5 * W1r).transpose(0, 2, 1).reshape(E, KT, P, F)
    w1s = np.ascontiguousarray(w1t.transpose(0, 2, 1, 3).astype(np.float16))
    # wf[e, p, k, c] = 16 * (W2r@W1r)[e].T, fp16
    wfr = 16.0 * np.einsum('ecf,efd->ecd', W2r, W1r, optimize=True)  # [E,C,C]
    wft = wfr.transpose(0, 2, 1).reshape(E, KT, P, C)
    wfs = np.ascontiguousarray(wft.transpose(0, 2, 1, 3).astype(np.float16))
    # w2[e, p, q, i, c] = fp8(32 * W2r[e].T)
    w2t = (SW * W2r).transpose(0, 2, 1).reshape(E, QT, KI, P, C)
    w2s = np.ascontiguousarray(w2t.transpose(0, 3, 1, 2, 4).astype(_E4))
    # b2' = b2 + SHIFT * rowsum(W2)
    b2p = np.ascontiguousarray(b2r + SHIFT * W2r.sum(axis=2))
    b1c = np.ascontiguousarray(b1r)

    in_maps = []
    for i in range(NCORES):
        es = slice(i * EPC, (i + 1) * EPC)
        in_maps.append({
            "xs": np.ascontiguousarray(xf[:, es]),
            "w1": np.ascontiguousarray(w1s[es]),
            "wf": np.ascontiguousarray(wfs[es]),
            "w2": np.ascontiguousarray(w2s[es]),
            "b1s": b1c[es],
            "b2s": b2p[es],
        })
    return in_maps


def run(x, W1, b1, W2, b2, trace=False, **trace_kwargs):
    nc = _get_nc()
    in_maps = _shard_inputs(x, W1, b1, W2, b2)
    res = run_bass_kernel_spmd(
        nc, in_maps, core_ids=list(range(NCORES)), trace=trace, **trace_kwargs
    )
    y = np.concatenate([res.results[i]["ys"] for i in range(NCORES)], axis=1)
    return y.astype(np.float32), res


def kernel(x, W1, b1, W2, b2):
    y, _ = run(x, W1, b1, W2, b2)
    return y


# revision 12
# speedup vs baseline: 1.6536x; 1.0216x over previous
"""Expert-parallel grouped-MLP (MoE experts) kernel for 8 Trainium2 cores.

Problem: y = W2_e @ silu(W1_e @ x_e + b1_e) + b2_e for E=16 independent
experts (grouped 1x1 conv), B=8 batches, C=256 channels/expert, CAP=4,
L=1024 positions. Expert-parallel: core i owns experts {2i, 2i+1}.

Speed trick ("linear hoist + single-pass fp8 residual path"):
  silu(z) = 0.5*z + g(z),  g = silu(z) - 0.5*z  (sigma_g ~ 0.45*sigma_h)
  y = W2@g + Wf@x + b2,    Wf := 0.5*(W2@W1)  (fused [C,C], host-exact)
The g-path runs as SINGLE fp8e4m3 DoubleRow matmuls (256-contraction per
pass -> 2x fp16 FLOP rate); g's small amplitude keeps the fp8
quantization error of both g and W2 inside the 2e-2 gate (measured
1.53e-2 on the fixed seed-0 inputs; plain fp8 h-path would be 5.3e-2).
The f-path and layer 1 stay fp16/exact. 28 512-col PE passes per
(pair, n-half) vs 32 for pure fp16.

Per (b, e) pair on-device:
  L1: per m-tile (8): 4 fp16 matmuls -> psum1 [128,1024] (= 0.5*z)
      ACT: h16 = silu(2*psum1 + b1)
      DVE/GpSimd (alternating): s8 = (h16 - 0.25) - psum1  -> fp8 (= g-0.25)
  L2: per (j,n): 2 fp16 Wf-matmuls + 4 fp8-DR W2g-matmuls -> psum2
      DVE: y16 = psum2/32 + b2'   (b2' = b2 + 0.25*rowsum(W2), host)
Host pre-scales: W1 x0.5 (psum holds 0.5z), W2g x32 fp8, Wf x16 fp16;
x ships fp16, y returns fp16 (upcast on host).
"""
import numpy as np
import ml_dtypes

import concourse.tile as tile
from concourse import bacc, mybir
from concourse.bass_utils import run_bass_kernel_spmd

# Problem constants (hardcoded per contract)
B, E, C, CAP, L = 8, 16, 256, 4, 1024
F = C * CAP            # 1024 hidden per expert
NCORES = 8
EPC = E // NCORES      # 2 experts per core
P = 128                # partitions
KT = C // P            # 2 fp16 k-tiles (layer-1 / f-path contraction)
KI = 2                 # DoubleRow k-interleave (256-contraction)
MT = F // P            # 8 m-tiles
JT = C // P            # 2 j-tiles
QT = F // (KI * P)     # 4 DoubleRow k-pairs (g-path contraction)
NT = L // 512          # 2 n-tiles of 512 cols
N_WARMUP = 16
SW = 32.0              # W2 scale
SHIFT = 0.25           # g mean shift (folded into b2')
GPS_M = 0              # m-tiles whose s8 runs via DVE-evac + GpSimd sub

_FP32 = mybir.dt.float32
_FP16 = mybir.dt.float16
_FP8 = mybir.dt.float8e4
_E4 = ml_dtypes.float8_e4m3


def _build():
    nc = bacc.Bacc("TRN2", target_bir_lowering=False, debug=False)
    DR = mybir.MatmulPerfMode.DoubleRow
    Silu = mybir.ActivationFunctionType.Silu
    Ident = mybir.ActivationFunctionType.Identity
    Sub = mybir.AluOpType.subtract
    Mult = mybir.AluOpType.mult
    Add = mybir.AluOpType.add

    # host layouts (contiguous per partition):
    #   xf[b, e, p, k, l] = fp16(x[b, e, k*128+p, l])
    #   w1[e, p, k, f]    = fp16(0.5 * W1r[e, f, k*128+p])
    #   wf[e, p, k, c]    = fp16(16 * (W2r@W1r)[e, c, k*128+p])
    #   w2[e, p, q, i, c] = fp8(32 * W2r[e, c, q*256+i*128+p])
    xs_d = nc.dram_tensor("xs", [B, EPC, P, KT, L], _FP16, kind="ExternalInput")
    w1_d = nc.dram_tensor("w1", [EPC, P, KT, F], _FP16, kind="ExternalInput")
    wf_d = nc.dram_tensor("wf", [EPC, P, KT, C], _FP16, kind="ExternalInput")
    w2_d = nc.dram_tensor("w2", [EPC, P, QT, KI, C], _FP8, kind="ExternalInput")
    b1_d = nc.dram_tensor("b1s", [EPC, F], _FP32, kind="ExternalInput")
    b2_d = nc.dram_tensor("b2s", [EPC, C], _FP32, kind="ExternalInput")
    ys_d = nc.dram_tensor("ys", [B, EPC * C, L], _FP16, kind="ExternalOutput")

    with tile.TileContext(nc) as tc:
        with (
            tc.tile_pool(name="const", bufs=1) as cpool,
            tc.tile_pool(name="x", bufs=6) as xpool,
            tc.tile_pool(name="h", bufs=2) as hpool,
            tc.tile_pool(name="y", bufs=4) as ypool,
            tc.tile_pool(name="ps1", bufs=3, space="PSUM") as ps1,
            tc.tile_pool(name="ps2", bufs=2, space="PSUM") as ps2,
        ):
            # ---- PE warmup: zero bf16 matmuls with no DMA deps ----
            wdum = cpool.tile([P, P], mybir.dt.bfloat16, tag="wdum")
            rdum = cpool.tile([P, 512], mybir.dt.bfloat16, tag="rdum")
            nc.vector.memset(wdum[:], 0.0)
            nc.vector.memset(rdum[:], 0.0)
            actdum = cpool.tile([P, 1], _FP32, tag="actdum")
            nc.scalar.activation(actdum[:], rdum[:, :1], Silu, bias=0.0)
            shiftc = cpool.tile([P, 1], _FP32, tag="shiftc")
            nc.vector.memset(shiftc[:], SHIFT)
            for i in range(N_WARMUP):
                pdum = ps2.tile([P, 512], _FP32, tag="ps2")
                nc.tensor.matmul(pdum[:], wdum[:], rdum[:],
                                 start=True, stop=True)

            # ---- weight/bias tiles ----
            w1sb = [cpool.tile([P, KT, F], _FP16, tag=f"w1_{e}",
                               name=f"w1sb_{e}") for e in range(EPC)]
            wfsb = [cpool.tile([P, KT, C], _FP16, tag=f"wf_{e}",
                               name=f"wfsb_{e}") for e in range(EPC)]
            w2sb = [cpool.tile([P, QT, KI, C], _FP8, tag=f"w2_{e}",
                               name=f"w2sb_{e}") for e in range(EPC)]
            b1sb = cpool.tile([P, EPC * MT], _FP32, tag="b1")  # col e*MT+m
            b2sb = cpool.tile([P, EPC * JT], _FP32, tag="b2")  # col e*JT+j

            def load_w(e):
                nc.sync.dma_start(w1sb[e][:], w1_d.ap()[e])
                nc.sync.dma_start(wfsb[e][:], wf_d.ap()[e])
                nc.sync.dma_start(w2sb[e][:], w2_d.ap()[e])

            def load_b(e):
                nc.sync.dma_start(
                    b1sb[:, e * MT:(e + 1) * MT],
                    b1_d.ap()[e].rearrange("(m p) -> p m", p=P),
                )
                nc.sync.dma_start(
                    b2sb[:, e * JT:(e + 1) * JT],
                    b2_d.ap()[e].rearrange("(j p) -> p j", p=P),
                )

            def load_x(b, e):
                # [P, KT, L] fp16; split per k-tile so mm0 waits on 2KB/part
                xt = xpool.tile([P, KT, L], _FP16, tag="x", name=f"x_{b}_{e}")
                for k in range(KT):
                    nc.sync.dma_start(xt[:, k], xs_d.ap()[b, e, :, k])
                return xt

            # startup-critical order
            load_b(0)
            nc.sync.dma_start(w1sb[0][:], w1_d.ap()[0])
            x0 = load_x(0, 0)
            nc.sync.dma_start(wfsb[0][:], wf_d.ap()[0])
            nc.sync.dma_start(w2sb[0][:], w2_d.ap()[0])

            # ---- per-(expert, batch) pipeline ----
            for e in range(EPC):
                for b in range(B):
                    xt = x0 if (e == 0 and b == 0) else load_x(b, e)
                    if e == 0 and b == 1:
                        load_w(1)
                        load_b(1)

                    # layer 1: psum1 = 0.5*z; h16 = silu(2*psum1 + b1)
                    h16 = hpool.tile([P, MT, L], _FP16, tag="h16",
                                     name=f"h16_{e}_{b}")
                    s8 = hpool.tile([P, MT, L], _FP8, tag="s8",
                                    name=f"s8_{e}_{b}")
                    for m in range(MT):
                        psm = ps1.tile([P, L], _FP32, tag="ps1",
                                       name=f"ps1_{e}_{b}_{m}")
                        for k in range(KT):
                            for n in range(NT):
                                nc.tensor.matmul(
                                    psm[:, n * 512:(n + 1) * 512],
                                    w1sb[e][:, k, m * P:(m + 1) * P],
                                    xt[:, k, n * 512:(n + 1) * 512],
                                    start=(k == 0), stop=(k == KT - 1))
                        nc.scalar.activation(
                            h16[:, m], psm[:], Silu,
                            bias=b1sb[:, e * MT + m: e * MT + m + 1],
                            scale=2.0)
                        # s8 = (h16 - SHIFT) - psum1   (= g - SHIFT, fp8)
                        if m < GPS_M:
                            # GpSimd can't read PSUM: DVE evacuates
                            # t16 = psum + SHIFT, GpSimd subtracts in SBUF
                            t16 = hpool.tile([P, L], _FP16, tag=f"t{m}",
                                             name=f"t16_{e}_{b}_{m}")
                            nc.vector.tensor_scalar(t16[:], psm[:],
                                                    float(SHIFT), None, op0=Add)
                            nc.gpsimd.tensor_sub(s8[:, m], h16[:, m], t16[:])
                        else:
                            nc.vector.scalar_tensor_tensor(
                                s8[:, m], h16[:, m], SHIFT, psm[:],
                                op0=Sub, op1=Sub)

                    # layer 2: psum2 = Wf@x (fp16) + W2g@s8 (fp8 DR)
                    for j in range(JT):
                        psy = [ps2.tile([P, 512], _FP32, tag="ps2",
                                        name=f"ps2_{e}_{b}_{j}_{n}")
                               for n in range(NT)]
                        for k in range(KT):
                            for n in range(NT):
                                nc.tensor.matmul(
                                    psy[n][:],
                                    wfsb[e][:, k, j * P:(j + 1) * P],
                                    xt[:, k, n * 512:(n + 1) * 512],
                                    start=(k == 0), stop=False)
                        for q in range(QT):
                            for n in range(NT):
                                nc.tensor.matmul(
                                    psy[n][:],
                                    w2sb[e][:, q, :, j * P:(j + 1) * P],
                                    s8[:, 2 * q:2 * q + 2,
                                       n * 512:(n + 1) * 512],
                                    start=False, stop=(q == QT - 1),
                                    perf_mode=DR)
                        for n in range(NT):
                            yt = ypool.tile([P, 512], _FP16, tag="y",
                                            name=f"y_{e}_{b}_{j}_{n}")
                            nc.scalar.activation(
                                yt[:], psy[n][:], Ident,
                                bias=b2sb[:, e * JT + j: e * JT + j + 1],
                                scale=1.0 / SW)
                            nc.sync.dma_start(
                                ys_d.ap()[b, e * C + j * P: e * C + (j + 1) * P,
                                          n * 512:(n + 1) * 512],
                                yt[:])

    nc.compile()
    return nc


_NC_CACHE = None


def _get_nc():
    global _NC_CACHE
    if _NC_CACHE is None:
        _NC_CACHE = _build()
    return _NC_CACHE


def _shard_inputs(x, W1, b1, W2, b2):
    """Full inputs -> list of 8 per-core input dicts (expert-parallel)."""
    x = np.ascontiguousarray(x, dtype=np.float32)
    # xf[b, e, p, k, l]
    xf = np.ascontiguousarray(
        x.reshape(B, E, KT, P, L).transpose(0, 1, 3, 2, 4).astype(np.float16))

    W1r = W1.astype(np.float32).reshape(E, F, C)
    W2r = W2.astype(np.float32).reshape(E, C, F)
    b1r = b1.astype(np.float32).reshape(E, F)
    b2r = b2.astype(np.float32).reshape(E, C)

    # w1[e, p, k, f] = 0.5 * W1r[e].T, fp16
    w1t = (0.5 * W1r).transpose(0, 2, 1).reshape(E, KT, P, F)
    w1s = np.ascontiguousarray(w1t.transpose(0, 2, 1, 3).astype(np.float16))
    # wf[e, p, k, c] = 16 * (W2r@W1r)[e].T, fp16
    wfr = 16.0 * np.einsum('ecf,efd->ecd', W2r, W1r, optimize=True)  # [E,C,C]
    wft = wfr.transpose(0, 2, 1).reshape(E, KT, P, C)
    wfs = np.ascontiguousarray(wft.transpose(0, 2, 1, 3).astype(np.float16))
    # w2[e, p, q, i, c] = fp8(32 * W2r[e].T)
    w2t = (SW * W2r).transpose(0, 2, 1).reshape(E, QT, KI, P, C)
    w2s = np.ascontiguousarray(w2t.transpose(0, 3, 1, 2, 4).astype(_E4))
    # b2' = b2 + SHIFT * rowsum(W2)
    b2p = np.ascontiguousarray(b2r + SHIFT * W2r.sum(axis=2))
    b1c = np.ascontiguousarray(b1r)

    in_maps = []
    for i in range(NCORES):
        es = slice(i * EPC, (i + 1) * EPC)
        in_maps.append({
            "xs": np.ascontiguousarray(xf[:, es]),
            "w1": np.ascontiguousarray(w1s[es]),
            "wf": np.ascontiguousarray(wfs[es]),
            "w2": np.ascontiguousarray(w2s[es]),
            "b1s": b1c[es],
            "b2s": b2p[es],
        })
    return in_maps


def run(x, W1, b1, W2, b2, trace=False, **trace_kwargs):
    nc = _get_nc()
    in_maps = _shard_inputs(x, W1, b1, W2, b2)
    res = run_bass_kernel_spmd(
        nc, in_maps, core_ids=list(range(NCORES)), trace=trace, **trace_kwargs
    )
    y = np.concatenate([res.results[i]["ys"] for i in range(NCORES)], axis=1)
    return y.astype(np.float32), res


def kernel(x, W1, b1, W2, b2):
    y, _ = run(x, W1, b1, W2, b2)
    return y


# revision 13
# speedup vs baseline: 1.7162x; 1.0378x over previous
"""Expert-parallel grouped-MLP (MoE experts) kernel for 8 Trainium2 cores.

Problem: y = W2_e @ silu(W1_e @ x_e + b1_e) + b2_e for E=16 independent
experts (grouped 1x1 conv), B=8 batches, C=256 channels/expert, CAP=4,
L=1024 positions. Expert-parallel: core i owns experts {2i, 2i+1}.

Speed trick ("linear hoist + single-pass fp8 residual path"):
  silu(z) = 0.5*z + g(z),  g = silu(z) - 0.5*z  (sigma_g ~ 0.45*sigma_h)
  y = W2@g + Wf@x + b2,    Wf := 0.5*(W2@W1)  (fused [C,C], host-exact)
The g-path runs as SINGLE fp8e4m3 DoubleRow matmuls (256-contraction per
pass -> 2x fp16 FLOP rate); g's small amplitude keeps the fp8
quantization error of both g and W2 inside the 2e-2 gate (measured
1.53e-2 on the fixed seed-0 inputs; plain fp8 h-path would be 5.3e-2).
The f-path and layer 1 stay fp16/exact. 28 512-col PE passes per
(pair, n-half) vs 32 for pure fp16.

Per (b, e) pair on-device:
  L1: per m-tile (8): 4 fp16 matmuls -> psum1 [128,1024] (= 0.5*z)
      ACT: h16 = silu(2*psum1 + b1)
      DVE/GpSimd (alternating): s8 = (h16 - 0.25) - psum1  -> fp8 (= g-0.25)
  L2: per (j,n): 2 fp16 Wf-matmuls + 4 fp8-DR W2g-matmuls -> psum2
      DVE: y16 = psum2/32 + b2'   (b2' = b2 + 0.25*rowsum(W2), host)
Host pre-scales: W1 x0.5 (psum holds 0.5z), W2g x32 fp8, Wf x16 fp16;
x ships fp16, y returns fp16 (upcast on host).
"""
import numpy as np
import ml_dtypes

import concourse.tile as tile
from concourse import bacc, mybir
from concourse.bass_utils import run_bass_kernel_spmd

# Problem constants (hardcoded per contract)
B, E, C, CAP, L = 8, 16, 256, 4, 1024
F = C * CAP            # 1024 hidden per expert
NCORES = 8
EPC = E // NCORES      # 2 experts per core
P = 128                # partitions
KT = C // P            # 2 fp16 k-tiles (layer-1 / f-path contraction)
KI = 2                 # DoubleRow k-interleave (256-contraction)
MT = F // P            # 8 m-tiles
JT = C // P            # 2 j-tiles
QT = F // (KI * P)     # 4 DoubleRow k-pairs (g-path contraction)
NT = L // 512          # 2 n-tiles of 512 cols
N_WARMUP = 16
SW = 32.0              # W2 scale
SHIFT = 0.25           # g mean shift (folded into b2')
GPS_M = 0              # m-tiles whose s8 runs via DVE-evac + GpSimd sub

_FP32 = mybir.dt.float32
_FP16 = mybir.dt.float16
_FP8 = mybir.dt.float8e4
_E4 = ml_dtypes.float8_e4m3


def _build():
    nc = bacc.Bacc("TRN2", target_bir_lowering=False, debug=False)
    DR = mybir.MatmulPerfMode.DoubleRow
    Silu = mybir.ActivationFunctionType.Silu
    Ident = mybir.ActivationFunctionType.Identity
    Sub = mybir.AluOpType.subtract
    Mult = mybir.AluOpType.mult
    Add = mybir.AluOpType.add

    # host layouts (contiguous per partition):
    #   xf[b, e, p, k, l] = fp16(x[b, e, k*128+p, l])
    #   w1[e, p, k, f]    = fp16(0.5 * W1r[e, f, k*128+p])
    #   wf[e, p, k, c]    = fp16(16 * (W2r@W1r)[e, c, k*128+p])
    #   w2[e, p, q, i, c] = fp8(32 * W2r[e, c, q*256+i*128+p])
    xs_d = nc.dram_tensor("xs", [B, EPC, P, KT, L], _FP16, kind="ExternalInput")
    w1_d = nc.dram_tensor("w1", [EPC, P, KT, F], _FP16, kind="ExternalInput")
    wf_d = nc.dram_tensor("wf", [EPC, P, KT, C], _FP16, kind="ExternalInput")
    w2_d = nc.dram_tensor("w2", [EPC, P, QT, KI, C], _FP8, kind="ExternalInput")
    b1_d = nc.dram_tensor("b1s", [EPC, F], _FP32, kind="ExternalInput")
    b2_d = nc.dram_tensor("b2s", [EPC, C], _FP32, kind="ExternalInput")
    ys_d = nc.dram_tensor("ys", [B, EPC * C, L], _FP16, kind="ExternalOutput")

    with tile.TileContext(nc) as tc:
        with (
            tc.tile_pool(name="const", bufs=1) as cpool,
            tc.tile_pool(name="x", bufs=6) as xpool,
            tc.tile_pool(name="h", bufs=2) as hpool,
            tc.tile_pool(name="y", bufs=4) as ypool,
            tc.tile_pool(name="ps1", bufs=3, space="PSUM") as ps1,
            tc.tile_pool(name="ps2", bufs=2, space="PSUM") as ps2,
        ):
            # ---- PE warmup: zero bf16 matmuls with no DMA deps ----
            wdum = cpool.tile([P, P], mybir.dt.bfloat16, tag="wdum")
            rdum = cpool.tile([P, 512], mybir.dt.bfloat16, tag="rdum")
            nc.vector.memset(wdum[:], 0.0)
            nc.vector.memset(rdum[:], 0.0)
            actdum = cpool.tile([P, 1], _FP32, tag="actdum")
            nc.scalar.activation(actdum[:], rdum[:, :1], Silu, bias=0.0)
            shiftc = cpool.tile([P, 1], _FP32, tag="shiftc")
            nc.vector.memset(shiftc[:], SHIFT)
            for i in range(N_WARMUP):
                pdum = ps2.tile([P, 512], _FP32, tag="ps2")
                nc.tensor.matmul(pdum[:], wdum[:], rdum[:],
                                 start=True, stop=True)

            # ---- weight/bias tiles ----
            w1sb = [cpool.tile([P, KT, F], _FP16, tag=f"w1_{e}",
                               name=f"w1sb_{e}") for e in range(EPC)]
            wfsb = [cpool.tile([P, KT, C], _FP16, tag=f"wf_{e}",
                               name=f"wfsb_{e}") for e in range(EPC)]
            w2sb = [cpool.tile([P, QT, KI, C], _FP8, tag=f"w2_{e}",
                               name=f"w2sb_{e}") for e in range(EPC)]
            b1sb = cpool.tile([P, EPC * MT], _FP32, tag="b1")  # col e*MT+m
            b2sb = cpool.tile([P, EPC * JT], _FP32, tag="b2")  # col e*JT+j

            def load_w(e):
                nc.sync.dma_start(w1sb[e][:], w1_d.ap()[e])
                nc.sync.dma_start(wfsb[e][:], wf_d.ap()[e])
                nc.sync.dma_start(w2sb[e][:], w2_d.ap()[e])

            def load_b(e):
                nc.sync.dma_start(
                    b1sb[:, e * MT:(e + 1) * MT],
                    b1_d.ap()[e].rearrange("(m p) -> p m", p=P),
                )
                nc.sync.dma_start(
                    b2sb[:, e * JT:(e + 1) * JT],
                    b2_d.ap()[e].rearrange("(j p) -> p j", p=P),
                )

            def load_x(b, e):
                # [P, KT, L] fp16; split per k-tile so mm0 waits on 2KB/part
                xt = xpool.tile([P, KT, L], _FP16, tag="x", name=f"x_{b}_{e}")
                for k in range(KT):
                    nc.sync.dma_start(xt[:, k], xs_d.ap()[b, e, :, k])
                return xt

            # startup-critical order (k-split so mm0 waits on ~384KB)
            load_b(0)
            nc.sync.dma_start(w1sb[0][:, 0], w1_d.ap()[0, :, 0])
            x0 = load_x(0, 0)
            nc.sync.dma_start(w1sb[0][:, 1], w1_d.ap()[0, :, 1])
            nc.sync.dma_start(wfsb[0][:], wf_d.ap()[0])
            nc.sync.dma_start(w2sb[0][:], w2_d.ap()[0])

            def emit_l1(e, b, xt, h16, s8, m_lo, m_hi):
                for m in range(m_lo, m_hi):
                    psm = ps1.tile([P, L], _FP32, tag="ps1",
                                   name=f"ps1_{e}_{b}_{m}")
                    for k in range(KT):
                        for n in range(NT):
                            nc.tensor.matmul(
                                psm[:, n * 512:(n + 1) * 512],
                                w1sb[e][:, k, m * P:(m + 1) * P],
                                xt[:, k, n * 512:(n + 1) * 512],
                                start=(k == 0), stop=(k == KT - 1))
                    nc.scalar.activation(
                        h16[:, m], psm[:], Silu,
                        bias=b1sb[:, e * MT + m: e * MT + m + 1],
                        scale=2.0)
                    # s8 = (h16 - SHIFT) - psum1   (= g - SHIFT, fp8)
                    nc.vector.scalar_tensor_tensor(
                        s8[:, m], h16[:, m], SHIFT, psm[:],
                        op0=Sub, op1=Sub)

            def emit_l2(e, b, xt, s8, j):
                psy = [ps2.tile([P, 512], _FP32, tag="ps2",
                                name=f"ps2_{e}_{b}_{j}_{n}")
                       for n in range(NT)]
                for k in range(KT):
                    for n in range(NT):
                        nc.tensor.matmul(
                            psy[n][:],
                            wfsb[e][:, k, j * P:(j + 1) * P],
                            xt[:, k, n * 512:(n + 1) * 512],
                            start=(k == 0), stop=False)
                for q in range(QT):
                    for n in range(NT):
                        nc.tensor.matmul(
                            psy[n][:],
                            w2sb[e][:, q, :, j * P:(j + 1) * P],
                            s8[:, 2 * q:2 * q + 2, n * 512:(n + 1) * 512],
                            start=False, stop=(q == QT - 1),
                            perf_mode=DR)
                for n in range(NT):
                    yt = ypool.tile([P, 512], _FP16, tag="y",
                                    name=f"y_{e}_{b}_{j}_{n}")
                    nc.scalar.activation(
                        yt[:], psy[n][:], Ident,
                        bias=b2sb[:, e * JT + j: e * JT + j + 1],
                        scale=1.0 / SW)
                    nc.sync.dma_start(
                        ys_d.ap()[b, e * C + j * P: e * C + (j + 1) * P,
                                  n * 512:(n + 1) * 512],
                        yt[:])

            # ---- software-pipelined pair loop: L2(k-1) interleaves L1(k) ----
            prev = None
            for e in range(EPC):
                for b in range(B):
                    xt = x0 if (e == 0 and b == 0) else load_x(b, e)
                    if e == 0 and b == 1:
                        load_w(1)
                        load_b(1)
                    h16 = hpool.tile([P, MT, L], _FP16, tag="h16",
                                     name=f"h16_{e}_{b}")
                    s8 = hpool.tile([P, MT, L], _FP8, tag="s8",
                                    name=f"s8_{e}_{b}")
                    emit_l1(e, b, xt, h16, s8, 0, MT // 2)
                    if prev is not None:
                        emit_l2(prev[0], prev[1], prev[2], prev[3], 0)
                    emit_l1(e, b, xt, h16, s8, MT // 2, MT)
                    if prev is not None:
                        emit_l2(prev[0], prev[1], prev[2], prev[3], 1)
                    prev = (e, b, xt, s8)
            emit_l2(prev[0], prev[1], prev[2], prev[3], 0)
            emit_l2(prev[0], prev[1], prev[2], prev[3], 1)

    nc.compile()
    return nc


_NC_CACHE = None


def _get_nc():
    global _NC_CACHE
    if _NC_CACHE is None:
        _NC_CACHE = _build()
    return _NC_CACHE


def _shard_inputs(x, W1, b1, W2, b2):
    """Full inputs -> list of 8 per-core input dicts (expert-parallel)."""
    x = np.ascontiguousarray(x, dtype=np.float32)
    # xf[b, e, p, k, l]
    xf = np.ascontiguousarray(
        x.reshape(B, E, KT, P, L).transpose(0, 1, 3, 2, 4).astype(np.float16))

    W1r = W1.astype(np.float32).reshape(E, F, C)
    W2r = W2.astype(np.float32).reshape(E, C, F)
    b1r = b1.astype(np.float32).reshape(E, F)
    b2r = b2.astype(np.float32).reshape(E, C)

    # w1[e, p, k, f] = 0.5 * W1r[e].T, fp16
    w1t = (0.5 * W1r).transpose(0, 2, 1).reshape(E, KT, P, F)
    w1s = np.ascontiguousarray(w1t.transpose(0, 2, 1, 3).astype(np.float16))
    # wf[e, p, k, c] = 16 * (W2r@W1r)[e].T, fp16
    wfr = 16.0 * np.einsum('ecf,efd->ecd', W2r, W1r, optimize=True)  # [E,C,C]
    wft = wfr.transpose(0, 2, 1).reshape(E, KT, P, C)
    wfs = np.ascontiguousarray(wft.transpose(0, 2, 1, 3).astype(np.float16))
    # w2[e, p, q, i, c] = fp8(32 * W2r[e].T)
    w2t = (SW * W2r).transpose(0, 2, 1).reshape(E, QT, KI, P, C)
    w2s = np.ascontiguousarray(w2t.transpose(0, 3, 1, 2, 4).astype(_E4))
    # b2' = b2 + SHIFT * rowsum(W2)
    b2p = np.ascontiguousarray(b2r + SHIFT * W2r.sum(axis=2))
    b1c = np.ascontiguousarray(b1r)

    in_maps = []
    for i in range(NCORES):
        es = slice(i * EPC, (i + 1) * EPC)
        in_maps.append({
            "xs": np.ascontiguousarray(xf[:, es]),
            "w1": np.ascontiguousarray(w1s[es]),
            "wf": np.ascontiguousarray(wfs[es]),
            "w2": np.ascontiguousarray(w2s[es]),
            "b1s": b1c[es],
            "b2s": b2p[es],
        })
    return in_maps


def run(x, W1, b1, W2, b2, trace=False, **trace_kwargs):
    nc = _get_nc()
    in_maps = _shard_inputs(x, W1, b1, W2, b2)
    res = run_bass_kernel_spmd(
        nc, in_maps, core_ids=list(range(NCORES)), trace=trace, **trace_kwargs
    )
    y = np.concatenate([res.results[i]["ys"] for i in range(NCORES)], axis=1)
    return y.astype(np.float32), res


def kernel(x, W1, b1, W2, b2):
    y, _ = run(x, W1, b1, W2, b2)
    return y


# revision 16
# speedup vs baseline: 1.7201x; 1.0023x over previous
"""Expert-parallel grouped-MLP (MoE experts) kernel for 8 Trainium2 cores.

Problem: y = W2_e @ silu(W1_e @ x_e + b1_e) + b2_e for E=16 independent
experts (grouped 1x1 conv), B=8 batches, C=256 channels/expert, CAP=4,
L=1024 positions. Expert-parallel: core i owns experts {2i, 2i+1}.

Speed trick ("linear hoist + single-pass fp8 residual path"):
  silu(z) = 0.5*z + g(z),  g = silu(z) - 0.5*z  (sigma_g ~ 0.45*sigma_h)
  y = W2@g + Wf@x + b2,    Wf := 0.5*(W2@W1)  (fused [C,C], host-exact)
The g-path runs as SINGLE fp8e4m3 DoubleRow matmuls (256-contraction per
pass -> 2x fp16 FLOP rate); g's small amplitude keeps the fp8
quantization error of both g and W2 inside the 2e-2 gate (measured
1.53e-2 on the fixed seed-0 inputs; plain fp8 h-path would be 5.3e-2).
The f-path and layer 1 stay fp16/exact. 28 512-col PE passes per
(pair, n-half) vs 32 for pure fp16.

Per (b, e) pair on-device:
  L1: per m-tile (8): 4 fp16 matmuls -> psum1 [128,1024] (= 0.5*z)
      ACT: h16 = silu(2*psum1 + b1)
      DVE/GpSimd (alternating): s8 = (h16 - 0.25) - psum1  -> fp8 (= g-0.25)
  L2: per (j,n): 2 fp16 Wf-matmuls + 4 fp8-DR W2g-matmuls -> psum2
      DVE: y16 = psum2/32 + b2'   (b2' = b2 + 0.25*rowsum(W2), host)
Host pre-scales: W1 x0.5 (psum holds 0.5z), W2g x32 fp8, Wf x16 fp16;
x ships fp16, y returns fp16 (upcast on host).
"""
import numpy as np
import ml_dtypes

import concourse.tile as tile
from concourse import bacc, mybir
from concourse.bass_utils import run_bass_kernel_spmd

# Problem constants (hardcoded per contract)
B, E, C, CAP, L = 8, 16, 256, 4, 1024
F = C * CAP            # 1024 hidden per expert
NCORES = 8
EPC = E // NCORES      # 2 experts per core
P = 128                # partitions
KT = C // P            # 2 fp16 k-tiles (layer-1 / f-path contraction)
KI = 2                 # DoubleRow k-interleave (256-contraction)
MT = F // P            # 8 m-tiles
JT = C // P            # 2 j-tiles
QT = F // (KI * P)     # 4 DoubleRow k-pairs (g-path contraction)
NT = L // 512          # 2 n-tiles of 512 cols
N_WARMUP = 16
SW = 32.0              # W2 scale
SHIFT = 0.25           # g mean shift (folded into b2')
GPS_M = 0              # m-tiles whose s8 runs via DVE-evac + GpSimd sub

_FP32 = mybir.dt.float32
_FP16 = mybir.dt.float16
_FP8 = mybir.dt.float8e4
_E4 = ml_dtypes.float8_e4m3


def _build():
    nc = bacc.Bacc("TRN2", target_bir_lowering=False, debug=False)
    DR = mybir.MatmulPerfMode.DoubleRow
    Silu = mybir.ActivationFunctionType.Silu
    Ident = mybir.ActivationFunctionType.Identity
    Sub = mybir.AluOpType.subtract
    Mult = mybir.AluOpType.mult
    Add = mybir.AluOpType.add

    # host layouts (contiguous per partition):
    #   xf[b, e, p, k, l] = fp16(x[b, e, k*128+p, l])
    #   w1[e, p, k, f]    = fp16(0.5 * W1r[e, f, k*128+p])
    #   wf[e, p, k, c]    = fp16(16 * (W2r@W1r)[e, c, k*128+p])
    #   w2[e, p, q, i, c] = fp8(32 * W2r[e, c, q*256+i*128+p])
    xs_d = nc.dram_tensor("xs", [B, EPC, P, KT, L], _FP16, kind="ExternalInput")
    w1_d = nc.dram_tensor("w1", [EPC, P, KT, F], _FP16, kind="ExternalInput")
    wf_d = nc.dram_tensor("wf", [EPC, P, KT, C], _FP16, kind="ExternalInput")
    w2_d = nc.dram_tensor("w2", [EPC, P, QT, KI, C], _FP8, kind="ExternalInput")
    b1_d = nc.dram_tensor("b1s", [EPC, F], _FP32, kind="ExternalInput")
    b2_d = nc.dram_tensor("b2s", [EPC, C], _FP32, kind="ExternalInput")
    ys_d = nc.dram_tensor("ys", [B, EPC * C, L], _FP16, kind="ExternalOutput")

    with tile.TileContext(nc) as tc:
        with (
            tc.tile_pool(name="const", bufs=1) as cpool,
            tc.tile_pool(name="x", bufs=6) as xpool,
            tc.tile_pool(name="h", bufs=2) as hpool,
            tc.tile_pool(name="y", bufs=4) as ypool,
            tc.tile_pool(name="ps1", bufs=3, space="PSUM") as ps1,
            tc.tile_pool(name="ps2", bufs=2, space="PSUM") as ps2,
        ):
            # ---- PE warmup: zero bf16 matmuls with no DMA deps ----
            wdum = cpool.tile([P, P], mybir.dt.bfloat16, tag="wdum")
            rdum = cpool.tile([P, 512], mybir.dt.bfloat16, tag="rdum")
            nc.vector.memset(wdum[:], 0.0)
            nc.vector.memset(rdum[:], 0.0)
            actdum = cpool.tile([P, 1], _FP32, tag="actdum")
            nc.scalar.activation(actdum[:], rdum[:, :1], Silu, bias=0.0)
            shiftc = cpool.tile([P, 1], _FP32, tag="shiftc")
            nc.vector.memset(shiftc[:], SHIFT)
            for i in range(N_WARMUP):
                pdum = ps2.tile([P, 512], _FP32, tag="ps2")
                nc.tensor.matmul(pdum[:], wdum[:], rdum[:],
                                 start=True, stop=True)

            # ---- weight/bias tiles ----
            w1sb = [cpool.tile([P, KT, F], _FP16, tag=f"w1_{e}",
                               name=f"w1sb_{e}") for e in range(EPC)]
            wfsb = [cpool.tile([P, KT, C], _FP16, tag=f"wf_{e}",
                               name=f"wfsb_{e}") for e in range(EPC)]
            w2sb = [cpool.tile([P, QT, KI, C], _FP8, tag=f"w2_{e}",
                               name=f"w2sb_{e}") for e in range(EPC)]
            b1sb = cpool.tile([P, EPC * MT], _FP32, tag="b1")  # col e*MT+m
            b2sb = cpool.tile([P, EPC * JT], _FP32, tag="b2")  # col e*JT+j

            def load_w(e):
                nc.sync.dma_start(w1sb[e][:], w1_d.ap()[e])
                nc.sync.dma_start(wfsb[e][:], wf_d.ap()[e])
                nc.sync.dma_start(w2sb[e][:], w2_d.ap()[e])

            def load_b(e):
                nc.sync.dma_start(
                    b1sb[:, e * MT:(e + 1) * MT],
                    b1_d.ap()[e].rearrange("(m p) -> p m", p=P),
                )
                nc.sync.dma_start(
                    b2sb[:, e * JT:(e + 1) * JT],
                    b2_d.ap()[e].rearrange("(j p) -> p j", p=P),
                )

            def load_x(b, e):
                # [P, KT, L] fp16; split per k-tile so mm0 waits on 2KB/part
                xt = xpool.tile([P, KT, L], _FP16, tag="x", name=f"x_{b}_{e}")
                for k in range(KT):
                    nc.sync.dma_start(xt[:, k], xs_d.ap()[b, e, :, k])
                return xt

            # startup-critical order (fine splits: mm0 waits on ~256KB)
            load_b(0)
            nc.sync.dma_start(w1sb[0][:, 0, 0:512], w1_d.ap()[0, :, 0, 0:512])
            x00 = xpool.tile([P, KT, L], _FP16, tag="x", name="x_0_0")
            nc.sync.dma_start(x00[:, 0, 0:512], xs_d.ap()[0, 0, :, 0, 0:512])
            nc.sync.dma_start(x00[:, 0, 512:L], xs_d.ap()[0, 0, :, 0, 512:L])
            nc.sync.dma_start(w1sb[0][:, 0, 512:F], w1_d.ap()[0, :, 0, 512:F])
            nc.sync.dma_start(x00[:, 1], xs_d.ap()[0, 0, :, 1])
            nc.sync.dma_start(w1sb[0][:, 1], w1_d.ap()[0, :, 1])
            nc.sync.dma_start(wfsb[0][:], wf_d.ap()[0])
            nc.sync.dma_start(w2sb[0][:], w2_d.ap()[0])
            x0 = x00

            def emit_l1(e, b, xt, h16, s8, m_lo, m_hi):
                for m in range(m_lo, m_hi):
                    psm = ps1.tile([P, L], _FP32, tag="ps1",
                                   name=f"ps1_{e}_{b}_{m}")
                    for k in range(KT):
                        for n in range(NT):
                            nc.tensor.matmul(
                                psm[:, n * 512:(n + 1) * 512],
                                w1sb[e][:, k, m * P:(m + 1) * P],
                                xt[:, k, n * 512:(n + 1) * 512],
                                start=(k == 0), stop=(k == KT - 1))
                    nc.scalar.activation(
                        h16[:, m], psm[:], Silu,
                        bias=b1sb[:, e * MT + m: e * MT + m + 1],
                        scale=2.0)
                    # s8 = (h16 - SHIFT) - psum1   (= g - SHIFT, fp8)
                    nc.vector.scalar_tensor_tensor(
                        s8[:, m], h16[:, m], SHIFT, psm[:],
                        op0=Sub, op1=Sub)

            def emit_l2_open(e, b, xt, s8, j, q_hi):
                psy = [ps2.tile([P, 512], _FP32, tag="ps2",
                                name=f"ps2_{e}_{b}_{j}_{n}")
                       for n in range(NT)]
                for k in range(KT):
                    for n in range(NT):
                        nc.tensor.matmul(
                            psy[n][:],
                            wfsb[e][:, k, j * P:(j + 1) * P],
                            xt[:, k, n * 512:(n + 1) * 512],
                            start=(k == 0), stop=False)
                for q in range(q_hi):
                    for n in range(NT):
                        nc.tensor.matmul(
                            psy[n][:],
                            w2sb[e][:, q, :, j * P:(j + 1) * P],
                            s8[:, 2 * q:2 * q + 2, n * 512:(n + 1) * 512],
                            start=False, stop=False, perf_mode=DR)
                return psy

            def emit_l2_close(e, b, s8, j, psy, q_lo):
                for q in range(q_lo, QT):
                    for n in range(NT):
                        nc.tensor.matmul(
                            psy[n][:],
                            w2sb[e][:, q, :, j * P:(j + 1) * P],
                            s8[:, 2 * q:2 * q + 2, n * 512:(n + 1) * 512],
                            start=False, stop=(q == QT - 1),
                            perf_mode=DR)
                for n in range(NT):
                    yt = ypool.tile([P, 512], _FP16, tag="y",
                                    name=f"y_{e}_{b}_{j}_{n}")
                    nc.scalar.activation(
                        yt[:], psy[n][:], Ident,
                        bias=b2sb[:, e * JT + j: e * JT + j + 1],
                        scale=1.0 / SW)
                    nc.sync.dma_start(
                        ys_d.ap()[b, e * C + j * P: e * C + (j + 1) * P,
                                  n * 512:(n + 1) * 512],
                        yt[:])

            def emit_l2(e, b, xt, s8, j):
                psy = emit_l2_open(e, b, xt, s8, j, QT - 1)
                emit_l2_close(e, b, s8, j, psy, QT - 1)

            # ---- software-pipelined pair loop: L2(k-1) interleaves L1(k) ----
            prev = None
            npairs = EPC * B
            for idx in range(npairs):
                e, b = idx // B, idx % B
                last = idx == npairs - 1
                xt = x0 if idx == 0 else load_x(b, e)
                if idx == 1:
                    load_w(1)
                    load_b(1)
                h16 = hpool.tile([P, MT, L], _FP16, tag="h16",
                                 name=f"h16_{e}_{b}")
                s8 = hpool.tile([P, MT, L], _FP8, tag="s8",
                                name=f"s8_{e}_{b}")
                emit_l1(e, b, xt, h16, s8, 0, 4)
                if prev is not None:
                    emit_l2(prev[0], prev[1], prev[2], prev[3], 0)
                if not last:
                    emit_l1(e, b, xt, h16, s8, 4, MT)
                    if prev is not None:
                        emit_l2(prev[0], prev[1], prev[2], prev[3], 1)
                else:
                    # tail-shortening: open own j0 (Wf + q0,q1) behind L1
                    emit_l1(e, b, xt, h16, s8, 4, 6)
                    emit_l2(prev[0], prev[1], prev[2], prev[3], 1)
                    psy0 = emit_l2_open(e, b, xt, s8, 0, 2)
                    emit_l1(e, b, xt, h16, s8, 6, MT)
                    emit_l2_close(e, b, s8, 0, psy0, 2)
                    emit_l2(e, b, xt, s8, 1)
                prev = (e, b, xt, s8)

    nc.compile()
    return nc


_NC_CACHE = None


def _get_nc():
    global _NC_CACHE
    if _NC_CACHE is None:
        _NC_CACHE = _build()
    return _NC_CACHE


def _shard_inputs(x, W1, b1, W2, b2):
    """Full inputs -> list of 8 per-core input dicts (expert-parallel)."""
    x = np.ascontiguousarray(x, dtype=np.float32)
    # xf[b, e, p, k, l]
    xf = np.ascontiguousarray(
        x.reshape(B, E, KT, P, L).transpose(0, 1, 3, 2, 4).astype(np.float16))

    W1r = W1.astype(np.float32).reshape(E, F, C)
    W2r = W2.astype(np.float32).reshape(E, C, F)
    b1r = b1.astype(np.float32).reshape(E, F)
    b2r = b2.astype(np.float32).reshape(E, C)

    # w1[e, p, k, f] = 0.5 * W1r[e].T, fp16
    w1t = (0.5 * W1r).transpose(0, 2, 1).reshape(E, KT, P, F)
    w1s = np.ascontiguousarray(w1t.transpose(0, 2, 1, 3).astype(np.float16))
    # wf[e, p, k, c] = 16 * (W2r@W1r)[e].T, fp16
    wfr = 16.0 * np.einsum('ecf,efd->ecd', W2r, W1r, optimize=True)  # [E,C,C]
    wft = wfr.transpose(0, 2, 1).reshape(E, KT, P, C)
    wfs = np.ascontiguousarray(wft.transpose(0, 2, 1, 3).astype(np.float16))
    # w2[e, p, q, i, c] = fp8(32 * W2r[e].T)
    w2t = (SW * W2r).transpose(0, 2, 1).reshape(E, QT, KI, P, C)
    w2s = np.ascontiguousarray(w2t.transpose(0, 3, 1, 2, 4).astype(_E4))
    # b2' = b2 + SHIFT * rowsum(W2)
    b2p = np.ascontiguousarray(b2r + SHIFT * W2r.sum(axis=2))
    b1c = np.ascontiguousarray(b1r)

    in_maps = []
    for i in range(NCORES):
        es = slice(i * EPC, (i + 1) * EPC)
        in_maps.append({
            "xs": np.ascontiguousarray(xf[:, es]),
            "w1": np.ascontiguousarray(w1s[es]),
            "wf": np.ascontiguousarray(wfs[es]),
            "w2": np.ascontiguousarray(w2s[es]),
            "b1s": b1c[es],
            "b2s": b2p[es],
        })
    return in_maps


def run(x, W1, b1, W2, b2, trace=False, **trace_kwargs):
    nc = _get_nc()
    in_maps = _shard_inputs(x, W1, b1, W2, b2)
    res = run_bass_kernel_spmd(
        nc, in_maps, core_ids=list(range(NCORES)), trace=trace, **trace_kwargs
    )
    y = np.concatenate([res.results[i]["ys"] for i in range(NCORES)], axis=1)
    return y.astype(np.float32), res


def kernel(x, W1, b1, W2, b2):
    y, _ = run(x, W1, b1, W2, b2)
    return y
